# revision 1
# baseline (speedup 1.0000x reference)
# Trainium2 Bass kernel for nn_LocalCrossAttentionModule.
#
# Math: softmax over a size-1 axis is identically 1, so q/k (and x_query,
# Wq, bq, Wk, bk) never affect the output. The module reduces to, per
# 5x5 patch p (576 of them = 4 batch x 12x12 grid, stride 36):
#   kvf_p  = flatten(x_kv patch)                  (3200,)
#   v_p    = Wv @ kvf_p + bv                      (1600,) viewed as (64, 5, 5)
#   z_p    = conv_w @ v_p[:, s] + conv_b          (128,) per pixel s in 5x5
# z_p is scattered into an otherwise-constant (conv_b) output map.
#
# Sharding: the 25 patch pixels s are split across 8 cores (4 slots each,
# 7 junk/dup slots). Every core sees all 576 patches as the matmul moving
# dim (2 chunks of 288 >= 256 keeps float32r matmuls at full rate).
# Host does layout only: patch gather, weight permutation/transpose,
# final scatter into the conv_b-filled canvas.

import numpy as np

B = 4
CKV = 128
HW_ = 432
E = 2
PP = 5          # patch side
STRIDE = 36
PI = 12         # patch grid side
NP = B * PI * PI   # 576 patches
KF = CKV * PP * PP  # 3200 kv features per patch
KT = KF // 128      # 25 contraction tiles
OUT = 64
O2 = 128
SLOTS = 4
M = SLOTS * OUT    # 256 v-features per core
NCHUNK = 288       # patch chunk (2 x 288 = 576)
NCORES = 8

DTYPE = "f16"      # "f32r" (most accurate) | "f16" (half DMA bytes, ~5e-4) | "bf16"

# pixel-slot assignment: cores 0-6 own 3 pixels (4th slot duplicates the
# first), core 7 owns 4.
S_LISTS = [[3 * c, 3 * c + 1, 3 * c + 2, 3 * c] for c in range(7)]
S_LISTS.append([21, 22, 23, 24])
VALID = [3] * 7 + [4]

_PROGRAM = {}


def _build_program(dtype=DTYPE):
    import concourse.mybir as mybir
    from concourse import bacc
    from concourse.tile import TileContext

    f32 = mybir.dt.float32
    half = {"bf16": mybir.dt.bfloat16, "f16": mybir.dt.float16}
    mm_dt = mybir.dt.float32r if dtype == "f32r" else half[dtype]
    # matmul-2 operand dtype: DVE cannot produce float32r, so f32r mode
    # runs the (tiny) second matmul in plain fp32
    v_dt = f32 if dtype == "f32r" else half[dtype]

    WKC = M + NP  # 832 cols per k-tile: [w(256) | kvf(576)]

    nc = bacc.Bacc()
    wk_d = nc.declare_dram_parameter("wk", [128, KT, WKC], mm_dt, isOutput=False)
    cwbc_d = nc.declare_dram_parameter("cwbc", [128, 131], f32, isOutput=False)
    z_d = nc.declare_dram_parameter("zout", [128, SLOTS, NP], f32, isOutput=True)

    with TileContext(nc) as tc:
        with (
            tc.tile_pool(name="consts", bufs=1) as cpool,
            tc.tile_pool(name="wbig", bufs=1) as wpool,
            tc.tile_pool(name="vbuf", bufs=1) as vpool,
            tc.tile_pool(name="zbuf", bufs=1) as zpool,
            tc.tile_pool(name="ps1", bufs=1, space="PSUM") as ps1,
            tc.tile_pool(name="ps2", bufs=3, space="PSUM") as ps2,
            tc.tile_pool(name="ps0", bufs=1, space="PSUM") as ps0,
        ):
            # PE warm-up: dummy matmuls on a zeroed scratch tile keep the
            # PE_HAM activity window busy from t~0 so real matmuls run at
            # 2.4 GHz instead of the cold 1.2 GHz
            warm_t = cpool.tile([128, 512], f32, name="warm_t")
            nc.gpsimd.memset(warm_t[:], 0.0)
            wps = ps0.tile([128, 512], f32, name="wps")
            for _ in range(4):
                nc.tensor.matmul(
                    wps[:], lhsT=warm_t[:, 0:128], rhs=warm_t[:],
                    start=True, stop=True,
                )

            cwbc_t = cpool.tile([128, 131], f32, name="cwbc_t")
            nc.sync.dma_start(cwbc_t[:], cwbc_d[:])
            # DVE-produced copy of conv_w.T so matmul-2 waits only on DVE
            cw_t = cpool.tile([128, 128], v_dt, name="cw_t")
            nc.vector.tensor_copy(cw_t[:], cwbc_t[:, 0:128])

            wk_t = wpool.tile([128, KT, WKC], mm_dt, name="wk_t")
            # chunked loads, small first so the first matmul starts early
            sizes = [1, 2, 3, 3, 4, 4, 4, 4]
            lo = 0
            for sz in sizes:
                nc.sync.dma_start(wk_t[:, lo:lo + sz, :], wk_d[:, lo:lo + sz, :])
                lo += sz

            # matmul 1: V[f, n] = sum_j WvT[j, f] * KVF_T[j, n]
            ps_v = [
                [ps1.tile([128, NCHUNK], f32, name=f"psv{m}{n}") for n in range(2)]
                for m in range(2)
            ]
            for k in range(KT):
                for m in range(2):
                    for n in range(2):
                        nc.tensor.matmul(
                            ps_v[m][n][:],
                            lhsT=wk_t[:, k, m * 128:(m + 1) * 128],
                            rhs=wk_t[:, k, M + n * NCHUNK:M + (n + 1) * NCHUNK],
                            start=(k == 0),
                            stop=(k == KT - 1),
                        )
                # keep-warm filler: PE would otherwise idle waiting for the
                # next k-tile DMA, letting PE_HAM throttle the clock to 1.2GHz.
                # Small moving dim: just enough activity to hold the clock.
                if k % 2 == 0:
                    nc.tensor.matmul(
                        wps[:, 0:128], lhsT=warm_t[:, 0:128],
                        rhs=warm_t[:, 0:128],
                        start=True, stop=True,
                    )

            # V to SBUF (+bv), zero-padded to 128 partitions per pixel-slot
            v_t = []
            for s in range(SLOTS):
                vt = vpool.tile([128, NP], v_dt, name=f"vt{s}")
                nc.vector.memset(vt[64:128, :], 0.0)
                v_t.append(vt)
            for m in range(2):
                for n in range(2):
                    for h in range(2):
                        s = 2 * m + h
                        nc.vector.tensor_tensor(
                            out=v_t[s][0:64, n * NCHUNK:(n + 1) * NCHUNK],
                            in0=ps_v[m][n][h * 64:(h + 1) * 64, :],
                            in1=cwbc_t[h * 64:(h + 1) * 64, 128 + m:129 + m]
                            .to_broadcast((64, NCHUNK)),
                            op=mybir.AluOpType.add,
                        )

            # matmul 2: z[o2, n] = sum_o conv_w[o2, o] * V[s*64+o, n]
            z_t = zpool.tile([128, SLOTS, NP], f32, name="z_t")
            for s in range(SLOTS):
                for n in range(2):
                    nsl = slice(n * NCHUNK, (n + 1) * NCHUNK)
                    psz = ps2.tile([128, NCHUNK], f32, name="psz")
                    nc.tensor.matmul(
                        psz[:],
                        lhsT=cw_t[:],
                        rhs=v_t[s][:, nsl],
                        start=True,
                        stop=True,
                    )
                    nc.vector.tensor_tensor(
                        out=z_t[:, s, nsl],
                        in0=psz[:],
                        in1=cwbc_t[:, 130:131].to_broadcast((128, NCHUNK)),
                        op=mybir.AluOpType.add,
                    )
                    # store each chunk as soon as it is ready
                    nc.sync.dma_start(z_d[:, s, nsl], z_t[:, s, nsl])
    nc.finalize()
    return nc


def _get_program(dtype=DTYPE):
    if dtype not in _PROGRAM:
        _PROGRAM[dtype] = _build_program(dtype)
    return _PROGRAM[dtype]


def _round_fp32r(a):
    """Round fp32 array to the FP32R grid (12-bit mantissa): (u+0x800)&~0xfff."""
    u = np.ascontiguousarray(a, dtype=np.float32).view(np.uint32)
    u = (u + np.uint32(0x800)) & np.uint32(0xFFFFF000)
    return u.view(np.float32)


def _mm_cast(a, dtype):
    if dtype == "f32r":
        return _round_fp32r(a)
    if dtype == "f16":
        return np.ascontiguousarray(a, dtype=np.float32).astype(np.float16)
    import ml_dtypes

    return np.ascontiguousarray(a, dtype=np.float32).astype(ml_dtypes.bfloat16)


def _prep_in_maps(x_kv, Wv, bv, conv_w, conv_b, dtype=DTYPE):
    """Host-side shard/layout prep. Returns list of per-core input dicts."""
    x_kv = np.ascontiguousarray(np.asarray(x_kv, dtype=np.float32))
    Wv = np.asarray(Wv, dtype=np.float32)
    bv = np.asarray(bv, dtype=np.float32)
    conv_w = np.asarray(conv_w, dtype=np.float32)
    conv_b = np.asarray(conv_b, dtype=np.float32)

    # gather all 5x5 patches (padded coords: top-left of patch (pi,pj) is
    # original coords (pi*36-2, pj*36-2))
    pad = np.zeros((B, CKV, HW_ + 2 * E, HW_ + 2 * E), np.float32)
    pad[:, :, E:HW_ + E, E:HW_ + E] = x_kv
    r = (np.arange(PI)[:, None] * STRIDE + np.arange(PP)).ravel()  # (60,)
    g = pad[:, :, r[:, None], r[None, :]]                # (B, C, 60, 60)
    g = g.reshape(B, CKV, PI, PP, PI, PP)
    # feature j = c*25 + pr*5 + pc ; patch n = b*144 + pi*12 + pj
    kvf_t = g.transpose(1, 3, 5, 0, 2, 4).reshape(KF, NP)   # (3200, 576)
    # device layout [partition, k-tile, patch]
    kvf_arr = kvf_t.reshape(KT, 128, NP).transpose(1, 0, 2)

    cw = np.zeros((128, 128), np.float32)
    cw[:OUT, :] = conv_w.T  # cw[o, o2] = conv_w[o2, o]

    in_maps = []
    for c in range(NCORES):
        perm = np.array(
            [o * PP * PP + s for s in S_LISTS[c] for o in range(OUT)], np.int64
        )
        wv_t = Wv[perm].T                      # (3200, 256)
        wv_arr = wv_t.reshape(KT, 128, M).transpose(1, 0, 2)
        # single blob: per k-tile [w(256) | kvf(576)]
        wk = np.concatenate([wv_arr, kvf_arr], axis=2)  # (128, 25, 832)
        wk = _mm_cast(wk, dtype)
        # f32 consts blob: [cw(128) | bv(2) | cb(1)]
        cwbc = np.empty((128, 131), np.float32)
        cwbc[:, 0:128] = cw
        cwbc[:, 128:130] = bv[perm].reshape(2, 128).T
        cwbc[:, 130] = conv_b
        in_maps.append({"wk": wk, "cwbc": cwbc})
    return in_maps


def _assemble(z_list, conv_b, out_dtype=np.float32):
    """Scatter per-core z outputs into the full (B, 128, 432, 432) map."""
    conv_b = np.asarray(conv_b, dtype=np.float32)
    y = np.empty((B, O2, HW_, HW_), np.float32)
    y[:] = conv_b.reshape(1, O2, 1, 1)
    base = np.arange(PI) * STRIDE
    for c in range(NCORES):
        z = z_list[c]  # (128, SLOTS, 576)
        for t in range(VALID[c]):
            s = S_LISTS[c][t]
            pr, pc = divmod(s, PP)
            blk = z[:, t, :].reshape(O2, B, PI, PI).transpose(1, 0, 2, 3)
            y[:, :, (base + pr)[:, None], (base + pc)[None, :]] = blk
    return y.astype(out_dtype, copy=False)


def _run(inputs, trace=False, trace_kwargs=None, dtype=DTYPE):
    from concourse.bass_utils import run_bass_kernel_spmd

    in_maps = _prep_in_maps(
        inputs["x_kv"], inputs["Wv"], inputs["bv"],
        inputs["conv_w"], inputs["conv_b"], dtype=dtype,
    )
    nc = _get_program(dtype)
    kw = {}
    if trace:
        kw["trace"] = True
        if trace_kwargs:
            kw.update(trace_kwargs)
    res = run_bass_kernel_spmd(nc, in_maps, list(range(NCORES)), **kw)
    z_list = [res.results[c]["zout"] for c in range(NCORES)]
    out = _assemble(z_list, inputs["conv_b"])
    return out, res


def kernel(**inputs):
    out, _ = _run(inputs, trace=False)
    return out



# revision 5
# speedup vs baseline: 1.0484x; 1.0484x over previous
# Trainium2 Bass kernel for nn_LocalCrossAttentionModule.
#
# Math: softmax over a size-1 axis is identically 1, so q/k (and x_query,
# Wq, bq, Wk, bk) never affect the output. The module reduces to, per
# 5x5 patch p (576 of them = 4 batch x 12x12 grid, stride 36):
#   kvf_p  = flatten(x_kv patch)                  (3200,)
#   v_p    = Wv @ kvf_p + bv                      (1600,) viewed as (64, 5, 5)
#   z_p    = conv_w @ v_p[:, s] + conv_b          (128,) per pixel s in 5x5
# z_p is scattered into an otherwise-constant (conv_b) output map.
#
# Biases are folded on the host: z = conv_w @ (Wv_s @ kvf) + cb_eff where
# cb_eff = conv_w @ bv_s + conv_b, so the device never touches bv.
#
# Sharding (8 cores = 4 pixel groups x 2 patch halves):
#   - pixel groups (7,6,6,6 of the 25 patch pixels) -> rows of Wv
#   - patch halves (288 patches each = 2 batches)   -> columns of kvf
# Per-core DMA: wk blob [128, 25, 736] f16 (448 w-cols + 288 kvf-cols per
# contraction tile) streamed k-major so matmuls trail the DMA, plus a tiny
# f32 consts blob. Output z [128, 7, 288] f16 per core.
# Host does layout only: patch gather, weight permutation, final scatter.

import numpy as np

B = 4
CKV = 128
HW_ = 432
E = 2
PP = 5            # patch side
STRIDE = 36
PI = 12           # patch grid side
NP = B * PI * PI  # 576 patches
KF = CKV * PP * PP  # 3200 kv features per patch
KT = KF // 128      # 25 contraction tiles
OUT = 64
O2 = 128
NCORES = 8

SLOTS = 7          # pixel slots per core (worst group has 7 pixels)
M = SLOTS * OUT    # 448 w columns
N = NP // 2        # 288 patches per core
WKC = M + N        # 736 cols per k-tile: [w(448) | kvf(288)]

# pixel groups: group g owns these pixels; 6-pixel groups duplicate their
# first pixel into slot 6 (computed but ignored by the host).
GROUPS = [
    [0, 1, 2, 3, 4, 5, 6],
    [7, 8, 9, 10, 11, 12, 7],
    [13, 14, 15, 16, 17, 18, 13],
    [19, 20, 21, 22, 23, 24, 19],
]
VALID = [7, 6, 6, 6]

# mm1 lhsT column chunks (partition rows of v)
MCH = [(0, 128), (128, 128), (256, 128), (384, 64)]
# tail slot processing order: slot 6 first (its v-copy runs on ACT in
# parallel with the DVE copies), then 0..5
SLOT_ORDER = [6, 0, 1, 2, 3, 4, 5]

NWARM = 40

_PROGRAM = {}


def _build_program():
    import concourse.mybir as mybir
    from concourse import bacc
    from concourse.tile import TileContext

    f32 = mybir.dt.float32
    f16 = mybir.dt.float16
    ident = mybir.ActivationFunctionType.Identity

    nc = bacc.Bacc()
    wk_d = nc.declare_dram_parameter("wk", [128, KT, WKC], f16, isOutput=False)
    cc_d = nc.declare_dram_parameter("cc", [128, 136], f32, isOutput=False)
    z_d = nc.declare_dram_parameter("zout", [128, SLOTS, N], f16, isOutput=True)

    with TileContext(nc) as tc:
        with (
            tc.tile_pool(name="consts", bufs=1) as cpool,
            tc.tile_pool(name="wbig", bufs=1) as wpool,
            tc.tile_pool(name="vbuf", bufs=1) as vpool,
            tc.tile_pool(name="zbuf", bufs=1) as zpool,
            tc.tile_pool(name="ps1", bufs=1, space="PSUM") as ps1,
            tc.tile_pool(name="ps2", bufs=3, space="PSUM") as ps2,
            tc.tile_pool(name="ps0", bufs=1, space="PSUM") as ps0,
        ):
            # ---- DMA issues first so both HWDGE rings start immediately
            wk_t = wpool.tile([128, KT, WKC], f16, name="wk_t")
            chunks = [(0, 1), (1, 2), (3, 2), (5, 3), (8, 3), (11, 4),
                      (15, 5), (20, 5)]
            for i, (lo, sz) in enumerate(chunks):
                eng = nc.sync if i % 2 == 0 else nc.scalar
                eng.dma_start(wk_t[:, lo:lo + sz, :], wk_d[:, lo:lo + sz, :])
            cc_t = cpool.tile([128, 136], f32, name="cc_t")
            nc.scalar.dma_start(cc_t[:], cc_d[:])

            # ---- PE warm-up: cheap f16 matmuls burn the cold-clock window
            # (~3.4us at 1.2GHz) before the first real matmul.
            warm_t = cpool.tile([128, 64], f16, name="warm_t")
            nc.gpsimd.memset(warm_t[:], 0.0)
            wps = ps0.tile([64, 64], f32, name="wps")
            for _ in range(NWARM):
                nc.tensor.matmul(
                    wps[:], lhsT=warm_t[:, 0:64], rhs=warm_t[:],
                    start=True, stop=True,
                )

            # DVE-produced f16 copy of conv_w.T for matmul 2 (duplicated in
            # both partition halves so lhsT base_partition matches rhs)
            cw16 = cpool.tile([128, 128], f16, name="cw16")
            nc.vector.tensor_copy(cw16[:], cc_t[:, 0:128])

            # ---- matmul 1: V[f, n] = sum_j W[j, f] * KVF[j, n], k-major
            psv = [ps1.tile([128, N], f32, name=f"psv{m}") for m in range(4)]
            for k in range(KT):
                for m, (c0, w) in enumerate(MCH):
                    nc.tensor.matmul(
                        psv[m][0:w, :],
                        lhsT=wk_t[:, k, c0:c0 + w],
                        rhs=wk_t[:, k, M:WKC],
                        start=(k == 0),
                        stop=(k == KT - 1),
                    )

            # ---- tail: per m-chunk copy V to SBUF (f16), then per pixel
            # slot mm2 + bias-add + store. Work split across DVE and ACT.
            v16 = [vpool.tile([128, N], f16, name=f"v16_{m}") for m in range(4)]
            nc.scalar.copy(v16[3][0:64, :], psv[3][0:64, :])
            for m in range(3):
                nc.vector.tensor_copy(v16[m][:], psv[m][:])

            for i, t in enumerate(SLOT_ORDER):
                m = t // 2
                p0 = 64 * (t % 2) if t < 6 else 0
                psz = ps2.tile([128, N], f32, name="psz")
                nc.tensor.matmul(
                    psz[:], lhsT=cw16[p0:p0 + 64, :],
                    rhs=v16[m][p0:p0 + 64, :],
                    start=True, stop=True,
                )
                z16 = zpool.tile([128, N], f16, name=f"z16_{t}")
                bias = cc_t[:, 128 + t:129 + t]
                if i % 2 == 0:
                    nc.scalar.activation(z16[:], psz[:], ident, bias=bias)
                else:
                    nc.vector.tensor_tensor(
                        out=z16[:], in0=psz[:],
                        in1=bias.to_broadcast((128, N)),
                        op=mybir.AluOpType.add,
                    )
                eng = nc.sync if i % 2 == 0 else nc.scalar
                eng.dma_start(z_d[:, t, :], z16[:])
    nc.finalize()
    return nc


def _get_program():
    if "p" not in _PROGRAM:
        _PROGRAM["p"] = _build_program()
    return _PROGRAM["p"]


def _prep_in_maps(x_kv, Wv, bv, conv_w, conv_b):
    """Host-side shard/layout prep. Returns list of per-core input dicts."""
    x_kv = np.ascontiguousarray(np.asarray(x_kv, dtype=np.float32))
    Wv = np.asarray(Wv, dtype=np.float32)
    bv = np.asarray(bv, dtype=np.float32)
    conv_w = np.asarray(conv_w, dtype=np.float32)
    conv_b = np.asarray(conv_b, dtype=np.float32)

    # gather all 5x5 patches (padded coords: top-left of patch (pi,pj) is
    # original coords (pi*36-2, pj*36-2))
    pad = np.zeros((B, CKV, HW_ + 2 * E, HW_ + 2 * E), np.float32)
    pad[:, :, E:HW_ + E, E:HW_ + E] = x_kv
    r = (np.arange(PI)[:, None] * STRIDE + np.arange(PP)).ravel()  # (60,)
    g = pad[:, :, r[:, None], r[None, :]]                # (B, C, 60, 60)
    g = g.reshape(B, CKV, PI, PP, PI, PP)
    # feature j = c*25 + pr*5 + pc ; patch n = b*144 + pi*12 + pj
    kvf_t = g.transpose(1, 3, 5, 0, 2, 4).reshape(KF, NP)   # (3200, 576)
    kvf_arrs = [
        np.ascontiguousarray(kvf_t[:, h * N:(h + 1) * N])
        .reshape(KT, 128, N).transpose(1, 0, 2)
        for h in range(2)
    ]

    wk_blobs = {}
    cc_blobs = {}
    for gi in range(4):
        perm = np.array(
            [o * PP * PP + s for s in GROUPS[gi] for o in range(OUT)], np.int64
        )
        wv_t = Wv[perm].T                       # (3200, 448)
        wv_arr = wv_t.reshape(KT, 128, M).transpose(1, 0, 2)
        for h in range(2):
            wk = np.concatenate([wv_arr, kvf_arrs[h]], axis=2)  # (128,25,736)
            wk_blobs[(gi, h)] = wk.astype(np.float16)
        cc = np.zeros((128, 136), np.float32)
        cc[0:64, 0:128] = conv_w.T              # cc[o, o2] = conv_w[o2, o]
        cc[64:128, 0:128] = conv_w.T            # duplicate for partition base
        for t in range(SLOTS):
            bv_slot = bv[perm[t * 64:(t + 1) * 64]]
            cc[:, 128 + t] = conv_w @ bv_slot + conv_b
        cc_blobs[gi] = cc

    in_maps = []
    for c in range(NCORES):
        gi, h = c // 2, c % 2
        in_maps.append({"wk": wk_blobs[(gi, h)], "cc": cc_blobs[gi]})
    return in_maps


def _assemble(z_list, conv_b, out_dtype=np.float32):
    """Scatter per-core z outputs into the full (B, 128, 432, 432) map."""
    conv_b = np.asarray(conv_b, dtype=np.float32)
    y = np.empty((B, O2, HW_, HW_), np.float32)
    y[:] = conv_b.reshape(1, O2, 1, 1)
    base = np.arange(PI) * STRIDE
    for c in range(NCORES):
        gi, h = c // 2, c % 2
        z = np.asarray(z_list[c], dtype=np.float32)  # (128, 7, 288)
        for t in range(VALID[gi]):
            s = GROUPS[gi][t]
            pr, pc = divmod(s, PP)
            blk = z[:, t, :].reshape(O2, 2, PI, PI).transpose(1, 0, 2, 3)
            y[2 * h:2 * h + 2, :, (base + pr)[:, None], (base + pc)[None, :]] = blk
    return y.astype(out_dtype, copy=False)


def _run(inputs, trace=False, trace_kwargs=None):
    from concourse.bass_utils import run_bass_kernel_spmd

    in_maps = _prep_in_maps(
        inputs["x_kv"], inputs["Wv"], inputs["bv"],
        inputs["conv_w"], inputs["conv_b"],
    )
    nc = _get_program()
    kw = {}
    if trace:
        kw["trace"] = True
        if trace_kwargs:
            kw.update(trace_kwargs)
    res = run_bass_kernel_spmd(nc, in_maps, list(range(NCORES)), **kw)
    z_list = [res.results[c]["zout"] for c in range(NCORES)]
    out = _assemble(z_list, inputs["conv_b"])
    return out, res


def kernel(**inputs):
    out, _ = _run(inputs, trace=False)
    return out
